# revision 12
# baseline (speedup 1.0000x reference)
"""Distributed GraphSAGE kernel for Trainium2 (8 NeuronCores, Bass/Tile) — v3.

Row-major aggregation scheme:
  - table row v (256B fp16): h_v @ Wl_next; self half maxdeg*(h_v @ Wr_next)
    kept SBUF-resident (never leaves the core)
  - gather (SWDGE dma_gather, elem 256B) fetches table rows of m[src] for
    each edge slot; 4 SWDGE queues, equal-size chunks, sliding-window issue
  - two passes per layer: pass A (table half 0) accumulates into SBUF
    partials; pass B (half 1) adds partials + self rows + bias and runs ACT.
    This hides the AllGather latency of half 1 behind pass A.
  - one-hot dst masks are generated ON DEVICE (DVE is_equal vs iota row)
    from a compact [128, NSUB] fp32 dst-rel tensor
  - per dst tile: PSUM[dst,f] += onehot_mask(lhsT) @ g(rhs) over subtiles,
    += ident @ partial, += ident @ self_rows, += maxdeg-row (x) bias-row
  - ACT: h = relu(PSUM * (1/maxdeg))  (per-partition scale = per-dst-node)
  - table build: hT via PE transpose, M = hT^T @ [Wl|Wr], agg half written
    to DRAM slab (batched, 4 tiles per DMA), AllGather (Shared) -> next
    table; self half scaled by maxdeg into SBUF selfbuf
  - layer 3: h3 = PSUM * invdeg (identity act), pooling via host-shipped
    per-tile graph one-hot/gsize masks: poolT[f,G] += h3(lhsT) @ gmask(rhs)
"""
import sys

sys.path.insert(0, "/opt/trn_rl_repo")

import numpy as np
from contextlib import ExitStack
from dataclasses import dataclass

from concourse import bass, mybir, tile, bacc
from concourse import bass_utils
from concourse.masks import make_identity

P = 128
IDXSPAN = 32768     # int16 index limit per gather chunk
F16 = mybir.dt.float16
F32 = mybir.dt.float32
I16 = mybir.dt.int16


@dataclass(frozen=True)
class Cfg:
    n_cores: int
    num_nodes: int
    num_edges: int
    in_feat: int
    hidden: int
    num_graphs: int
    num_classes: int
    n_own: int              # padded nodes per core (multiple of NB*128)
    nb: int                 # node tiles per gather batch
    gpc: int                # graphs per core
    k_tc: tuple             # per (tile, chunk) subtile counts, len NT*CH
    gmaxp: int              # max subtiles per (batch, pass)

    @property
    def nt(self):
        return self.n_own // P

    @property
    def half(self):
        return self.n_own // 2

    @property
    def hrows(self):
        return self.n_cores * self.half

    @property
    def ch_per_half(self):
        return -(-self.hrows // IDXSPAN)

    @property
    def csz(self):
        return -(-self.hrows // self.ch_per_half)

    @property
    def ch(self):
        return 2 * self.ch_per_half

    @property
    def nsub(self):
        return sum(self.k_tc)


def build_program(cfg: Cfg):
    nc = bacc.Bacc(
        "TRN2",
        target_bir_lowering=False,
        debug=False,
        num_devices=cfg.n_cores,
        num_swdge_queues=4,
    )

    NT, NB = cfg.nt, cfg.nb
    NSUB = cfg.nsub
    HID = cfg.hidden
    H2 = 2 * HID
    INF = cfg.in_feat
    GPC = cfg.gpc
    NC = cfg.n_cores
    CH = cfg.ch
    CPH = cfg.ch_per_half
    CSZ = cfg.csz
    HALF = cfg.half
    HROWS = cfg.hrows
    NBATCH = NT // NB
    AGA_BATCH = -(-(NT // 2) // NB) - 1   # after this batch, half-0 tiles done
    KTC = cfg.k_tc          # [t*CH + c]
    GMAXP = cfg.gmaxp
    WIN = 2                 # gather batches in flight per pass (8 lane cap)

    # subtile order (b, c, t, j); per (b,c) contiguous slot columns
    sub_of = {}             # (t, c, j) -> global subtile index
    s = 0
    call_meta = []          # per (b, c): (s0, k_bc)
    for b in range(NBATCH):
        for c in range(CH):
            s0 = s
            for ti in range(NB):
                t = b * NB + ti
                for j in range(KTC[t * CH + c]):
                    sub_of[(t, c, j)] = s
                    s += 1
            call_meta.append((s0, s - s0))
    assert s == NSUB

    # ---- I/O -------------------------------------------------------------
    xT_d = nc.dram_tensor("xT", [INF, cfg.n_own], F16, kind="ExternalInput")
    idx_d = nc.dram_tensor("eidx", [P, NSUB * 8], I16, kind="ExternalInput")
    drel_d = nc.dram_tensor("drel", [P, NSUB], F16, kind="ExternalInput")
    iota_d = nc.dram_tensor("iotar", [P, P], F16, kind="ExternalInput")
    invdeg_d = nc.dram_tensor("invdeg", [P, NT], F32, kind="ExternalInput")
    degrow_d = nc.dram_tensor("degrow", [1, cfg.n_own], F16, kind="ExternalInput")
    gmask_d = nc.dram_tensor("gmask", [P, NT * GPC], F16, kind="ExternalInput")
    degcol_d = nc.dram_tensor("degcol", [P, NT], F32, kind="ExternalInput")
    WlWr1_d = nc.dram_tensor("WlWr1", [INF, H2], F16, kind="ExternalInput")
    WlWr2_d = nc.dram_tensor("WlWr2", [HID, H2], F16, kind="ExternalInput")
    WlWr3_d = nc.dram_tensor("WlWr3", [HID, H2], F16, kind="ExternalInput")
    Wlin_d = nc.dram_tensor("Wlin", [HID, cfg.num_classes], F16, kind="ExternalInput")
    brow1_d = nc.dram_tensor("brow1", [1, HID], F16, kind="ExternalInput")
    brow2_d = nc.dram_tensor("brow2", [1, HID], F16, kind="ExternalInput")
    brow3_d = nc.dram_tensor("brow3", [1, HID], F16, kind="ExternalInput")
    blinrow_d = nc.dram_tensor("blinrow", [1, cfg.num_classes], F16, kind="ExternalInput")
    onesrow_d = nc.dram_tensor("onesrow", [1, GPC], F16, kind="ExternalInput")
    out_d = nc.dram_tensor("out", [cfg.num_classes, GPC], F32, kind="ExternalOutput")

    rg = [list(range(NC))]

    with tile.TileContext(nc) as tc, ExitStack() as ctx:
        sb = ctx.enter_context(tc.tile_pool(name="sb", bufs=1))
        mk = ctx.enter_context(tc.tile_pool(name="mk", bufs=3))
        hb = ctx.enter_context(tc.tile_pool(name="hb", bufs=4))
        hbm4 = ctx.enter_context(tc.tile_pool(name="hbm4", bufs=2))
        gbuf = ctx.enter_context(tc.tile_pool(name="gbuf", bufs=7))
        ps = ctx.enter_context(tc.tile_pool(name="ps", bufs=2, space="PSUM"))
        ps2 = ctx.enter_context(tc.tile_pool(name="ps2", bufs=2, space="PSUM"))
        pool_ps = ctx.enter_context(tc.tile_pool(name="pps", bufs=1, space="PSUM"))
        dram = ctx.enter_context(tc.tile_pool(name="dram", bufs=1, space="DRAM"))

        # ---- static SBUF state ------------------------------------------
        ident16 = sb.tile([P, P], F16)
        make_identity(nc, ident16[:])

        idx_sb = sb.tile([P, NSUB * 8], I16)
        nc.sync.dma_start(idx_sb[:], idx_d[:, :])
        drel_sb = sb.tile([P, NSUB], F16)
        nc.sync.dma_start(drel_sb[:], drel_d[:, :])
        iota_sb = sb.tile([P, P], F16)
        nc.sync.dma_start(iota_sb[:], iota_d[:, :])
        invdeg_sb = sb.tile([P, NT], F32)
        nc.sync.dma_start(invdeg_sb[:], invdeg_d[:, :])
        degrow_sb = sb.tile([1, cfg.n_own], F16)
        nc.sync.dma_start(degrow_sb[:], degrow_d[:, :])
        gmask_sb = sb.tile([P, NT * GPC], F16)
        nc.sync.dma_start(gmask_sb[:], gmask_d[:, :])
        degcol_sb = sb.tile([P, NT], F32)
        nc.sync.dma_start(degcol_sb[:], degcol_d[:, :])

        def load_w(d, p_, f_, nm):
            t = sb.tile([p_, f_], F16, name=nm, tag=nm)
            nc.sync.dma_start(t[:], d[:, :])
            return t

        WlWr1_sb = load_w(WlWr1_d, INF, H2, "w1s")
        WlWr2_sb = load_w(WlWr2_d, HID, H2, "w2s")
        WlWr3_sb = load_w(WlWr3_d, HID, H2, "w3s")
        Wlin_sb = load_w(Wlin_d, HID, cfg.num_classes, "wlins")
        brow1_sb = load_w(brow1_d, 1, HID, "b1s")
        brow2_sb = load_w(brow2_d, 1, HID, "b2s")
        brow3_sb = load_w(brow3_d, 1, HID, "b3s")
        blinrow_sb = load_w(blinrow_d, 1, cfg.num_classes, "bls")
        onesrow_sb = load_w(onesrow_d, 1, GPC, "o1s")

        partial_sb = sb.tile([P, NT * HID], F16, name="partial")
        selfbuf = sb.tile([P, NT * HID], F16, name="selfbuf")
        tc.no_sync_barrier()

        # ---- internal DRAM ----------------------------------------------
        slab_agg = dram.tile([cfg.n_own, HID], F16, tag="slaba", name="slaba")
        tbls = [[dram.tile([HROWS, HID], F16, tag=f"tbl{l}{h}",
                           name=f"tbl{l}{h}", addr_space="Shared")
                 for h in range(2)] for l in range(3)]

        def ag_half(lyr, h):
            nc.gpsimd.collective_compute(
                "AllGather", mybir.AluOpType.bypass, replica_groups=rg,
                ins=[slab_agg[h * HALF:(h + 1) * HALF, :]],
                outs=[tbls[lyr][h].opt()],
            )

        def slab_write(b, m4):
            nc.sync.dma_start(
                slab_agg[b * NB * P:(b + 1) * NB * P, :].rearrange(
                    "(t p) e -> p t e", p=P),
                m4[:].rearrange("p (t e) -> p t e", e=HID))

        def build_tile(ti, t, lhsT, W_sb, m4):
            """M = lhs @ [Wl|Wr]; agg half -> m4; self half * maxdeg -> selfbuf."""
            m_ps = ps2.tile([P, H2], F32, tag="mps")
            nc.tensor.matmul(out=m_ps[:], lhsT=lhsT, rhs=W_sb[:],
                             start=True, stop=True)
            nc.scalar.copy(m4[:, ti * HID:(ti + 1) * HID], m_ps[:, 0:HID])
            nc.scalar.mul(selfbuf[:, t * HID:(t + 1) * HID], m_ps[:, HID:H2],
                          degcol_sb[:, t:t + 1])

        # ---- P0: table1 = x @ [Wl1|Wr1] ---------------------------------
        xpool = ctx.enter_context(tc.tile_pool(name="xp", bufs=3))
        for b in range(NBATCH):
            xt = xpool.tile([INF, NB * P], F16, tag="xt")
            nc.sync.dma_start(xt[:], xT_d[:, b * NB * P:(b + 1) * NB * P])
            m4 = hbm4.tile([P, NB * HID], F16, tag="m4")
            for ti in range(NB):
                t = b * NB + ti
                build_tile(ti, t, xt[:, ti * P:(ti + 1) * P], WlWr1_sb, m4)
            slab_write(b, m4)
            if b == AGA_BATCH:
                ag_half(0, 0)
        ag_half(0, 1)

        # ---- layers ------------------------------------------------------
        def emit_pass_gathers(b, pss, tbl):
            c0 = pss * CPH
            g_t = gbuf.tile([P, GMAXP * P], F16, tag="g")
            off = 0
            offs = []
            for ci in range(CPH):
                c = c0 + ci
                s0, k_bc = call_meta[b * CH + c]
                offs.append(off)
                if k_bc == 0:
                    continue
                rlo = (c % CPH) * CSZ
                rhi = min(rlo + CSZ, HROWS)
                # split the call across two queues for drain overlap
                kh = [(k_bc + 1) // 2, k_bc // 2]
                so = s0
                for piece in range(2):
                    k_p = kh[piece]
                    if k_p == 0:
                        continue
                    nidx = k_p * P
                    nc.gpsimd.dma_gather(
                        out_ap=g_t[:, off * P:(off + k_p) * P].rearrange(
                            "p (t e) -> p t e", e=HID),
                        in_ap=tbl[c // CPH][rlo:rhi, :],
                        idxs_ap=idx_sb[:, so * 8:(so + k_p) * 8],
                        num_idxs=nidx,
                        num_idxs_reg=nidx,
                        elem_size=HID,
                        single_packet=False,
                        queue_num=2 * ci + piece,
                    )
                    off += k_p
                    so += k_p
            return g_t, offs

        for layer in range(3):
            tbl = tbls[layer]
            brow_sb = (brow1_sb, brow2_sb, brow3_sb)[layer]
            W_next = (WlWr2_sb, WlWr3_sb, None)[layer]

            if layer == 2:
                poolT_ps = pool_ps.tile([HID, GPC], F32, tag="pool")

            for pss in range(2):
                c0 = pss * CPH
                pend = {}
                for b in range(min(WIN, NBATCH)):
                    pend[b] = emit_pass_gathers(b, pss, tbl)
                for b in range(NBATCH):
                    g_t, offs = pend.pop(b)
                    if b + WIN < NBATCH:
                        pend[b + WIN] = emit_pass_gathers(b + WIN, pss, tbl)
                    bs0 = call_meta[b * CH + c0][0]
                    bw = sum(call_meta[b * CH + c0 + ci][1] for ci in range(CPH))
                    mk_t = mk.tile([P, GMAXP * P], F16, tag="mk")
                    if bw:
                        nc.vector.tensor_tensor(
                            out=mk_t[:, :bw * P].rearrange(
                                "p (a b) -> p a b", b=P),
                            in0=iota_sb[:].unsqueeze(1).broadcast_to(
                                [P, bw, P]),
                            in1=drel_sb[:, bs0:bs0 + bw].unsqueeze(-1)
                                .broadcast_to([P, bw, P]),
                            op=mybir.AluOpType.is_equal)

                    m4 = None
                    if pss == 1 and layer < 2:
                        m4 = hbm4.tile([P, NB * HID], F16, tag="m4")

                    for ti in range(NB):
                        t = b * NB + ti
                        # (mask col, gather col) pairs for this tile
                        pairs = []
                        for ci in range(CPH):
                            c = c0 + ci
                            pre = sum(KTC[(b * NB + u) * CH + c]
                                      for u in range(ti))
                            for j in range(KTC[t * CH + c]):
                                scol = sub_of[(t, c, j)]
                                pairs.append((scol - bs0, offs[ci] + pre + j))

                        if pss == 0:
                            if not pairs:
                                nc.vector.memset(
                                    partial_sb[:, t * HID:(t + 1) * HID], 0.0)
                                continue
                            out_ps = ps.tile([P, HID], F32, tag="agg")
                            for i, (mcol, gcol) in enumerate(pairs):
                                nc.tensor.matmul(
                                    out=out_ps[:],
                                    lhsT=mk_t[:, mcol * P:(mcol + 1) * P],
                                    rhs=g_t[:, gcol * P:(gcol + 1) * P],
                                    start=(i == 0), stop=(i == len(pairs) - 1))
                            nc.vector.tensor_copy(
                                partial_sb[:, t * HID:(t + 1) * HID], out_ps[:])
                            continue

                        # pass B: partial + msgs + self + bias -> ACT
                        out_ps = ps.tile([P, HID], F32, tag="agg")
                        nc.tensor.matmul(
                            out=out_ps[:], lhsT=ident16[:],
                            rhs=partial_sb[:, t * HID:(t + 1) * HID],
                            start=True, stop=False)
                        for (mcol, gcol) in pairs:
                            nc.tensor.matmul(
                                out=out_ps[:],
                                lhsT=mk_t[:, mcol * P:(mcol + 1) * P],
                                rhs=g_t[:, gcol * P:(gcol + 1) * P],
                                start=False, stop=False)
                        nc.tensor.matmul(
                            out=out_ps[:], lhsT=ident16[:],
                            rhs=selfbuf[:, t * HID:(t + 1) * HID],
                            start=False, stop=False)
                        nc.tensor.matmul(
                            out=out_ps[:],
                            lhsT=degrow_sb[:, t * P:(t + 1) * P],
                            rhs=brow_sb[:],
                            start=False, stop=True)

                        h_sb = hb.tile([P, HID], F16, tag="h")
                        nc.scalar.activation(
                            h_sb[:], out_ps[:],
                            (mybir.ActivationFunctionType.Relu if layer < 2
                             else mybir.ActivationFunctionType.Identity),
                            bias=0.0,
                            scale=invdeg_sb[:, t:t + 1],
                        )

                        if layer < 2:
                            hT_ps = ps2.tile([P, HID], F16, tag="htps")
                            nc.tensor.transpose(hT_ps[:], h_sb[:], ident16[:])
                            hT_sb = hb.tile([P, HID], F16, tag="htsb")
                            nc.scalar.copy(hT_sb[:], hT_ps[:])
                            build_tile(ti, t, hT_sb[:], W_next, m4)
                        else:
                            nc.tensor.matmul(
                                out=poolT_ps[:], lhsT=h_sb[:],
                                rhs=gmask_sb[:, t * GPC:(t + 1) * GPC],
                                start=(t == 0), stop=(t == NT - 1),
                            )

                    if pss == 1 and layer < 2:
                        slab_write(b, m4)
                        if b == AGA_BATCH:
                            ag_half(layer + 1, 0)
                    tc.no_sync_barrier()

                if pss == 1 and layer < 2:
                    ag_half(layer + 1, 1)

        # ---- head --------------------------------------------------------
        poolT_sb = sb.tile([HID, GPC], F16)
        nc.vector.tensor_copy(poolT_sb[:], poolT_ps[:])
        fin_ps = pool_ps.tile([cfg.num_classes, GPC], F32, tag="fin")
        nc.tensor.matmul(
            out=fin_ps[:], lhsT=Wlin_sb[:], rhs=poolT_sb[:],
            start=True, stop=False,
        )
        nc.tensor.matmul(
            out=fin_ps[:], lhsT=blinrow_sb[:], rhs=onesrow_sb[:],
            start=False, stop=True,
        )
        fin_sb = sb.tile([cfg.num_classes, GPC], F32)
        nc.vector.tensor_copy(fin_sb[:], fin_ps[:])
        nc.sync.dma_start(out_d[:, :], fin_sb[:])

    nc.compile()
    return nc


# --------------------------------------------------------------------------
# Host-side preprocessing
# --------------------------------------------------------------------------

def preprocess(x, edge_index, batch, cfg_overrides=None):
    num_nodes = x.shape[0]
    in_feat = x.shape[1]
    num_edges = edge_index.shape[1]
    batch = np.asarray(batch, dtype=np.int64)
    src_all = np.asarray(edge_index[0], dtype=np.int64)
    dst_all = np.asarray(edge_index[1], dtype=np.int64)
    n_cores = 8
    nb = 4
    num_graphs = (int(cfg_overrides.get("num_graphs"))
                  if cfg_overrides and "num_graphs" in cfg_overrides else 512)
    gpc = num_graphs // n_cores

    bounds = np.searchsorted(batch, np.arange(n_cores + 1) * gpc)
    nl = bounds[1:] - bounds[:-1]
    blk = nb * P
    n_own = int(-(-int(nl.max()) // blk) * blk)
    nt = n_own // P
    half = n_own // 2
    hrows = n_cores * half
    ch_per_half = -(-hrows // IDXSPAN)
    csz = -(-hrows // ch_per_half)
    n_ch = 2 * ch_per_half

    deg = np.bincount(dst_all, minlength=num_nodes)
    maxdeg = np.maximum(deg, 1).astype(np.float32)

    owner_d = (batch[dst_all] // gpc).astype(np.int64)
    owner_s = (batch[src_all] // gpc).astype(np.int64)
    rloc_full = (src_all - bounds[owner_s]).astype(np.int64)
    h_of = rloc_full // half
    row_in_half = owner_s * half + (rloc_full - h_of * half)
    ch_local = row_in_half // csz
    chunk = h_of * ch_per_half + ch_local
    src_rel = (row_in_half - ch_local * csz).astype(np.int16)
    ld = (dst_all - bounds[owner_d]).astype(np.int64)
    tile_of = ld // P

    # group key per edge: (core, tile, chunk)
    CH = n_ch
    gkey = (owner_d * nt + tile_of) * CH + chunk
    ngroups = n_cores * nt * CH
    gcounts = np.bincount(gkey, minlength=ngroups).reshape(n_cores, nt * CH)
    # shared budgets: max over cores of ceil(cnt/128)
    k_tc = tuple(int(v) for v in
                 np.ceil(gcounts.max(axis=0) / P).astype(np.int64))
    nsub = sum(k_tc)

    # global subtile index per (t, c): order (b, c, t, j) — mirror program
    nbatch = nt // nb
    sub_base = np.zeros(nt * CH, np.int64)   # first subtile idx of group (t,c)
    s = 0
    gmaxp = 0
    for b in range(nbatch):
        for pss in range(2):
            w = 0
            for c in range(pss * ch_per_half, (pss + 1) * ch_per_half):
                for ti in range(nb):
                    t = b * nb + ti
                    sub_base[t * CH + c] = s
                    s += k_tc[t * CH + c]
                    w += k_tc[t * CH + c]
            gmaxp = max(gmaxp, w)
    assert s == nsub

    # slot assignment: edges sorted by (core, gkey_local) then rank in group
    order = np.argsort(gkey, kind="stable")
    gk_sorted = gkey[order]
    gflat = np.bincount(gkey, minlength=ngroups)
    group_start = np.zeros(ngroups, np.int64)
    group_start[1:] = np.cumsum(gflat)[:-1]
    rank = np.arange(num_edges) - group_start[gk_sorted]
    core_s = gk_sorted // (nt * CH)
    tc_s = gk_sorted % (nt * CH)
    slot = sub_base[tc_s] * P + rank          # global flat slot within core

    e_proc = nsub * P
    idx_arr = np.zeros((n_cores, e_proc), np.int16)
    dstrel_arr = np.full((n_cores, e_proc), -1, np.int64)

    eo = order
    assert (rank < np.array(k_tc)[tc_s] * P).all(), "budget overflow"
    idx_arr[core_s, slot] = src_rel[eo]
    dstrel_arr[core_s, slot] = ld[eo] - tile_of[eo] * P

    def to_i16(a):
        band = a.reshape(e_proc // 16, 16).T
        return np.ascontiguousarray(np.tile(band, (8, 1)))

    iota_row = np.ascontiguousarray(
        np.tile(np.arange(P, dtype=np.float16), (P, 1)))

    per_core = []
    gsizes = np.bincount(batch, minlength=num_graphs).astype(np.float32)
    for c in range(n_cores):
        n0, n1 = int(bounds[c]), int(bounds[c + 1])
        ncore = n1 - n0
        xT = np.zeros((in_feat, n_own), np.float16)
        xT[:, :ncore] = x[n0:n1].T.astype(np.float16)

        md = np.zeros(n_own, np.float32)
        md[:ncore] = maxdeg[n0:n1]
        invdeg = np.ones(n_own, np.float32)
        invdeg[:ncore] = 1.0 / maxdeg[n0:n1]

        gm = np.zeros((n_own, gpc), np.float16)
        gb = (batch[n0:n1] - c * gpc).astype(np.int64)
        gs = gsizes[batch[n0:n1]]
        vals = np.zeros(ncore, np.float32)
        vals[gs > 0] = 1.0 / gs[gs > 0]
        gm[np.arange(ncore), gb] = vals.astype(np.float16)
        # [P, NT*GPC]: tile t cols [t*gpc:(t+1)*gpc], partition = node%128
        gm3 = gm.reshape(nt, P, gpc)

        per_core.append(dict(
            xT=xT,
            eidx=to_i16(idx_arr[c]),
            drel=np.ascontiguousarray(
                dstrel_arr[c].reshape(nsub, P).T.astype(np.float16)),
            iotar=iota_row,
            invdeg=np.ascontiguousarray(invdeg.reshape(nt, P).T),
            degrow=np.ascontiguousarray(md.astype(np.float16).reshape(1, n_own)),
            degcol=np.ascontiguousarray(md.reshape(nt, P).T),
            gmask=np.ascontiguousarray(
                gm3.transpose(1, 0, 2).reshape(P, nt * gpc)),
        ))

    cfg = Cfg(
        n_cores=n_cores, num_nodes=num_nodes, num_edges=num_edges,
        in_feat=in_feat, hidden=128, num_graphs=num_graphs,
        num_classes=2, n_own=n_own, nb=nb, gpc=gpc, k_tc=k_tc,
        gmaxp=gmaxp,
    )
    return cfg, per_core


def make_in_maps(cfg, per_core, weights):
    wmap = {}
    for i in (1, 2, 3):
        wl = np.asarray(weights[f"Wl{i}"], np.float32)
        wr = np.asarray(weights[f"Wr{i}"], np.float32)
        wmap[f"WlWr{i}"] = np.ascontiguousarray(
            np.concatenate([wl, wr], axis=1).astype(np.float16))
        wmap[f"brow{i}"] = np.ascontiguousarray(
            np.asarray(weights[f"bl{i}"], np.float32)
            .astype(np.float16).reshape(1, -1))
    wmap["Wlin"] = np.ascontiguousarray(
        np.asarray(weights["Wlin"], np.float32).astype(np.float16))
    wmap["blinrow"] = np.ascontiguousarray(
        np.asarray(weights["blin"], np.float32).astype(np.float16).reshape(1, -1))
    wmap["onesrow"] = np.ones((1, cfg.gpc), np.float16)
    in_maps = []
    for c in range(cfg.n_cores):
        m = dict(per_core[c])
        m.update(wmap)
        in_maps.append(m)
    return in_maps


_PROGRAM_CACHE = {}


def kernel(x, edge_index, batch,
           Wl1, bl1, Wr1, Wl2, bl2, Wr2, Wl3, bl3, Wr3, Wlin, blin):
    x = np.asarray(x)
    cfg, per_core = preprocess(np.asarray(x, np.float32),
                               np.asarray(edge_index), np.asarray(batch))
    weights = dict(Wl1=np.asarray(Wl1), bl1=np.asarray(bl1), Wr1=np.asarray(Wr1),
                   Wl2=np.asarray(Wl2), bl2=np.asarray(bl2), Wr2=np.asarray(Wr2),
                   Wl3=np.asarray(Wl3), bl3=np.asarray(bl3), Wr3=np.asarray(Wr3),
                   Wlin=np.asarray(Wlin), blin=np.asarray(blin))
    in_maps = make_in_maps(cfg, per_core, weights)

    key = (cfg.n_own, cfg.k_tc, cfg.in_feat, cfg.num_graphs)
    if key not in _PROGRAM_CACHE:
        _PROGRAM_CACHE[key] = build_program(cfg)
    nc = _PROGRAM_CACHE[key]

    res = bass_utils.run_bass_kernel_spmd(
        nc, in_maps, core_ids=list(range(cfg.n_cores)),
    )
    out = np.empty((cfg.num_graphs, cfg.num_classes), np.float32)
    for c in range(cfg.n_cores):
        out[c * cfg.gpc:(c + 1) * cfg.gpc, :] = res.results[c]["out"].T
    return out
